# revision 1
# baseline (speedup 1.0000x reference)
"""DQT (dequantized-ternary) linear layer on 8 Trainium2 NeuronCores.

Computation: w = (ternary * group_scales) in fp32; out = x @ w.T
  x:       (2, 4096, 4096) fp32
  ternary: (4096, 4096) int8 in {-1, 0, 1}
  scales:  (131072,) fp32, one per contiguous group of 128 weights
  out:     (2, 4096, 4096) fp32

Sharding (8 cores): 2-way data-parallel over tokens x 4-way tensor-parallel
over out_features. Each core dequantizes its weight shard on-chip
(int8 x broadcast scale row -> float32r) and runs a K=4096 PSUM-accumulated
matmul with float32r (e8m11) operands, full PE rate at N=512.

Host-side prep is layout only: transpose/tile x for the contraction-on-
partitions matmul layout, round fp32 operands to the e8m11 grid the PE's
FP32R mode uses, and slice the shards.
"""

import numpy as np

import concourse.bass as bass
import concourse.mybir as mybir
import concourse.tile as tile
from concourse.bass_utils import run_bass_kernel_spmd

F32 = mybir.dt.float32
F32R = mybir.dt.float32r
I8 = mybir.dt.int8

# Problem shape (hardcoded per harness contract)
B, S, K, O = 2, 4096, 4096, 4096
GS = 128
DP, TP = 2, 4  # data-parallel x tensor-parallel grid over the 8 cores
M = B * S
M_c, O_c = M // DP, O // TP
KT, MT, OC = K // 128, M_c // 128, O_c // 512

_nc_cache = {}


def _round_f32r(x: np.ndarray) -> np.ndarray:
    """Round fp32 to e8m11 (the FP32R grid): keep top 20 bits, RNE."""
    u = np.ascontiguousarray(x).view(np.uint32)
    r = (u + np.uint32(0x7FF) + ((u >> np.uint32(12)) & np.uint32(1))) & np.uint32(
        0xFFFFF000
    )
    return r.view(np.float32)


def _split_excess_waits(nc, cap: int = 1) -> None:
    """This walrus build fits at most one sync-wait in most instruction
    structs ("Too many sync wait commands"). Hoist excess waits into
    same-engine NoOps placed just before the instruction; engine streams
    are FIFO so semantics are unchanged."""
    for bb in nc.m.functions[0].blocks:
        out = []
        for ins in bb.instructions:
            si = ins.sync_info
            w = list(si.on_wait) if si and si.on_wait else []
            if len(w) > cap:
                for j, wd in enumerate(w[:-cap]):
                    nop = mybir.InstNoOp(
                        name=f"{ins.name}-wait{j}", ins=[], outs=[],
                        engine=ins.engine,
                    )
                    nop.sync_info = mybir.SyncInfo(on_wait=[wd], on_update=[])
                    out.append(nop)
                ins.sync_info = mybir.SyncInfo(
                    on_wait=w[-cap:], on_update=list(si.on_update or [])
                )
            out.append(ins)
        bb.instructions = out


def _build_nc():
    nc = bass.Bass()
    # x pre-tiled on host: [MT, 128, KT*128]; per-partition rows contiguous
    xT_d = nc.dram_tensor("xT", [MT, 128, KT * 128], F32R, kind="ExternalInput")
    ternT_d = nc.dram_tensor("ternT", [K, O_c], I8, kind="ExternalInput")
    scalesT_d = nc.dram_tensor("scalesT", [KT, O_c], F32, kind="ExternalInput")
    out_d = nc.dram_tensor("out", [M_c, O_c], F32, kind="ExternalOutput")

    with tile.TileContext(nc) as tc:
        with (
            tc.tile_pool(name="wp", bufs=1) as wpool,
            tc.tile_pool(name="dq", bufs=3) as dqpool,
            tc.tile_pool(name="xp", bufs=2) as xpool,
            tc.tile_pool(name="op", bufs=2) as opool,
            tc.tile_pool(name="ps", bufs=3, space="PSUM") as pspool,
        ):
            # prefetch first x tiles ahead of the dequant DMA burst, split
            # into chunks across both HWDGE engines to cut arrival latency
            xts = {}
            for mi in range(2):
                xt_pre = xpool.tile([128, KT * 128], F32R, tag="x")
                W = KT * 128
                nch = 4 if mi == 0 else 2
                for c in range(nch):
                    eng = nc.sync if c % 2 == 0 else nc.scalar
                    sl = slice(c * W // nch, (c + 1) * W // nch)
                    eng.dma_start(xt_pre[:, sl], xT_d[mi][:, sl])
                xts[mi] = xt_pre

            # dequant prologue: wT[k] = ternT[k-block] * scales (f32r).
            # Dequant DMAs go via the ACT HWDGE so the x-tile loads on the
            # SP HWDGE don't queue behind the scale-broadcast traffic.
            wts = []
            for k in range(KT):
                tt = dqpool.tile([128, O_c], I8, tag="tern")
                nc.scalar.dma_start(tt[:], ternT_d[k * 128 : (k + 1) * 128, :])
                sb = dqpool.tile([128, O_c], F32, tag="scale")
                nc.scalar.dma_start(
                    sb[:], scalesT_d[k : k + 1, :].broadcast_to([128, O_c])
                )
                wt = wpool.tile([128, O_c], F32R, tag=f"w{k}")
                # 20:12 DVE:GpSimd split (GpSimd TT is ~1.7x slower)
                eng = nc.gpsimd if k % 8 in (2, 5, 7) else nc.vector
                eng.tensor_mul(wt[:], tt[:], sb[:])
                wts.append(wt)

            # main loop: out[mi-block] = x-tile.T @ wT, K accumulated in PSUM
            for mi in range(MT):
                if mi in xts:
                    xt = xts[mi]
                else:
                    xt = xpool.tile([128, KT * 128], F32R, tag="x")
                    nc.sync.dma_start(xt[:], xT_d[mi])
                ps = pspool.tile([128, OC * 512], F32, tag="ps")
                for oc in range(OC):
                    for k in range(KT):
                        nc.tensor.matmul(
                            ps[:, oc * 512 : (oc + 1) * 512],
                            xt[:, k * 128 : (k + 1) * 128],
                            wts[k][:, oc * 512 : (oc + 1) * 512],
                            start=(k == 0),
                            stop=(k == KT - 1),
                        )
                ob = opool.tile([128, O_c], F32, tag="ob")
                nc.vector.tensor_copy(ob[:], ps[:])
                nc.sync.dma_start(out_d[mi * 128 : (mi + 1) * 128, :], ob[:])

    _split_excess_waits(nc)
    return nc


def _host_prep(x2d, ternary, scales):
    ternT = np.ascontiguousarray(ternary.T)  # [K, O] int8
    scalesT = _round_f32r(
        np.ascontiguousarray(scales.reshape(O, KT).T)
    )  # [KT, O]
    xr = _round_f32r(x2d)
    xtiles = []
    for dp in range(DP):
        xs = xr[dp * M_c : (dp + 1) * M_c]  # [M_c, K]
        t = np.ascontiguousarray(
            xs.reshape(MT, 128, KT, 128).transpose(0, 3, 2, 1)
        ).reshape(MT, 128, KT * 128)
        # t[mi, p, k*128+j] = xs[mi*128 + j, k*128 + p]
        xtiles.append(t)
    in_maps = []
    for c in range(DP * TP):
        dp, tp = divmod(c, TP)
        in_maps.append(
            {
                "xT": xtiles[dp],
                "ternT": np.ascontiguousarray(ternT[:, tp * O_c : (tp + 1) * O_c]),
                "scalesT": np.ascontiguousarray(
                    scalesT[:, tp * O_c : (tp + 1) * O_c]
                ),
            }
        )
    return in_maps


def kernel(x, ternary, scales, _trace=False):
    x = np.asarray(x, dtype=np.float32)
    ternary = np.asarray(ternary).astype(np.int8)  # {-1, 0, 1}
    scales = np.asarray(scales, dtype=np.float32)
    assert x.shape == (B, S, K) and ternary.shape == (O, K)

    if "nc" not in _nc_cache:
        _nc_cache["nc"] = _build_nc()
    nc = _nc_cache["nc"]

    in_maps = _host_prep(x.reshape(M, K), ternary, scales)
    res = run_bass_kernel_spmd(nc, in_maps, list(range(DP * TP)), trace=_trace)

    out2d = np.empty((M, O), np.float32)
    for c in range(DP * TP):
        dp, tp = divmod(c, TP)
        out2d[dp * M_c : (dp + 1) * M_c, tp * O_c : (tp + 1) * O_c] = res.results[
            c
        ]["out"]
    out = out2d.reshape(B, S, O)
    if _trace:
        return out, res.exec_time_ns
    return out


# revision 3
# speedup vs baseline: 1.0476x; 1.0476x over previous
"""DQT (dequantized-ternary) linear layer on 8 Trainium2 NeuronCores.

Computation: w = (ternary * group_scales) in fp32; out = x @ w.T
  x:       (2, 4096, 4096) fp32
  ternary: (4096, 4096) int8 in {-1, 0, 1}
  scales:  (131072,) fp32, one per contiguous group of 128 weights
  out:     (2, 4096, 4096) fp32

Sharding (8 cores): 2-way data-parallel over tokens x 4-way tensor-parallel
over out_features. Each core dequantizes its weight shard on-chip
(int8 x broadcast scale row -> float32r) and runs a K=4096 PSUM-accumulated
matmul with float32r (e8m11) operands, full PE rate at N=512.

Host-side prep is layout only: transpose/tile x for the contraction-on-
partitions matmul layout, round fp32 operands to the e8m11 grid the PE's
FP32R mode uses, and slice the shards.
"""

import numpy as np

import concourse.bass as bass
import concourse.mybir as mybir
import concourse.tile as tile
from concourse.bass_utils import run_bass_kernel_spmd

F32 = mybir.dt.float32
F32R = mybir.dt.float32r
I8 = mybir.dt.int8

# Problem shape (hardcoded per harness contract)
B, S, K, O = 2, 4096, 4096, 4096
GS = 128
DP, TP = 2, 4  # data-parallel x tensor-parallel grid over the 8 cores
M = B * S
M_c, O_c = M // DP, O // TP
KT, MT, OC = K // 128, M_c // 128, O_c // 512

_nc_cache = {}


def _round_f32r(x: np.ndarray) -> np.ndarray:
    """Round fp32 to e8m11 (the FP32R grid): keep top 20 bits, RNE."""
    u = np.ascontiguousarray(x).view(np.uint32)
    r = (u + np.uint32(0x7FF) + ((u >> np.uint32(12)) & np.uint32(1))) & np.uint32(
        0xFFFFF000
    )
    return r.view(np.float32)


def _split_excess_waits(nc, cap: int = 1) -> None:
    """This walrus build fits at most one sync-wait in most instruction
    structs ("Too many sync wait commands"). Hoist excess waits into
    same-engine NoOps placed just before the instruction; engine streams
    are FIFO so semantics are unchanged."""
    for bb in nc.m.functions[0].blocks:
        out = []
        for ins in bb.instructions:
            si = ins.sync_info
            w = list(si.on_wait) if si and si.on_wait else []
            if len(w) > cap:
                for j, wd in enumerate(w[:-cap]):
                    nop = mybir.InstNoOp(
                        name=f"{ins.name}-wait{j}", ins=[], outs=[],
                        engine=ins.engine,
                    )
                    nop.sync_info = mybir.SyncInfo(on_wait=[wd], on_update=[])
                    out.append(nop)
                ins.sync_info = mybir.SyncInfo(
                    on_wait=w[-cap:], on_update=list(si.on_update or [])
                )
            out.append(ins)
        bb.instructions = out


def _build_nc():
    nc = bass.Bass()
    # x pre-tiled on host: [MT, 128, KT*128]; per-partition rows contiguous
    xT_d = nc.dram_tensor("xT", [MT, 128, KT * 128], F32R, kind="ExternalInput")
    ternT_d = nc.dram_tensor("ternT", [K, O_c], I8, kind="ExternalInput")
    scalesT_d = nc.dram_tensor("scalesT", [KT, O_c], F32, kind="ExternalInput")
    out_d = nc.dram_tensor("out", [M_c, O_c], F32, kind="ExternalOutput")

    with tile.TileContext(nc) as tc:
        with (
            tc.tile_pool(name="wp", bufs=1) as wpool,
            tc.tile_pool(name="dq", bufs=3) as dqpool,
            tc.tile_pool(name="xp", bufs=3) as xpool,
            tc.tile_pool(name="op", bufs=2) as opool,
            tc.tile_pool(name="ps", bufs=3, space="PSUM") as pspool,
        ):
            # prefetch first x tiles ahead of the dequant DMA burst, split
            # into chunks across both HWDGE engines to cut arrival latency
            PRE = 3
            xts = {}
            for mi in range(PRE):
                xt_pre = xpool.tile([128, KT * 128], F32R, tag="x")
                W = KT * 128
                nch = 4 if mi == 0 else 2
                for c in range(nch):
                    eng = nc.sync if c % 2 == 0 else nc.scalar
                    sl = slice(c * W // nch, (c + 1) * W // nch)
                    eng.dma_start(xt_pre[:, sl], xT_d[mi][:, sl])
                xts[mi] = xt_pre

            # dequant prologue: wT[k] = ternT[k-block] * scales (f32r).
            # Dequant DMAs go via the ACT HWDGE so the x-tile loads on the
            # SP HWDGE don't queue behind the scale-broadcast traffic.
            wts = []
            for k in range(KT):
                tt = dqpool.tile([128, O_c], I8, tag="tern")
                nc.scalar.dma_start(tt[:], ternT_d[k * 128 : (k + 1) * 128, :])
                sb = dqpool.tile([128, O_c], F32, tag="scale")
                nc.scalar.dma_start(
                    sb[:], scalesT_d[k : k + 1, :].broadcast_to([128, O_c])
                )
                wt = wpool.tile([128, O_c], F32R, tag=f"w{k}")
                # 20:12 DVE:GpSimd split (GpSimd TT is ~1.7x slower)
                eng = nc.gpsimd if k % 8 in (2, 5, 7) else nc.vector
                eng.tensor_mul(wt[:], tt[:], sb[:])
                wts.append(wt)

            def emit_epilogue(mi, ps):
                ob = opool.tile([128, O_c], F32, tag="ob")
                nc.vector.tensor_copy(ob[:], ps[:])
                nc.sync.dma_start(out_d[mi * 128 : (mi + 1) * 128, :], ob[:])

            # first PRE m-tiles: interleave their accumulation chains at the
            # k level so each wT[k] (produced at dequant pace) feeds 2*PRE
            # back-to-back matmuls instead of 2 — PE is strict FIFO, so
            # chain-major order would stall on every fresh wT[k]
            pss = [
                pspool.tile([128, OC * 512], F32, tag="ps", name=f"ps{i}")
                for i in range(PRE)
            ]
            for k in range(KT):
                for mi in range(PRE):
                    for oc in range(OC):
                        nc.tensor.matmul(
                            pss[mi][:, oc * 512 : (oc + 1) * 512],
                            xts[mi][:, k * 128 : (k + 1) * 128],
                            wts[k][:, oc * 512 : (oc + 1) * 512],
                            start=(k == 0),
                            stop=(k == KT - 1),
                        )
            for mi in range(PRE):
                emit_epilogue(mi, pss[mi])

            # steady state: out[mi-block] = x-tile.T @ wT, K accum in PSUM
            for mi in range(PRE, MT):
                xt = xpool.tile([128, KT * 128], F32R, tag="x")
                nc.sync.dma_start(xt[:], xT_d[mi])
                ps = pspool.tile([128, OC * 512], F32, tag="ps")
                for oc in range(OC):
                    for k in range(KT):
                        nc.tensor.matmul(
                            ps[:, oc * 512 : (oc + 1) * 512],
                            xt[:, k * 128 : (k + 1) * 128],
                            wts[k][:, oc * 512 : (oc + 1) * 512],
                            start=(k == 0),
                            stop=(k == KT - 1),
                        )
                emit_epilogue(mi, ps)

    _split_excess_waits(nc)
    return nc


def _host_prep(x2d, ternary, scales):
    ternT = np.ascontiguousarray(ternary.T)  # [K, O] int8
    scalesT = _round_f32r(
        np.ascontiguousarray(scales.reshape(O, KT).T)
    )  # [KT, O]
    xr = _round_f32r(x2d)
    xtiles = []
    for dp in range(DP):
        xs = xr[dp * M_c : (dp + 1) * M_c]  # [M_c, K]
        t = np.ascontiguousarray(
            xs.reshape(MT, 128, KT, 128).transpose(0, 3, 2, 1)
        ).reshape(MT, 128, KT * 128)
        # t[mi, p, k*128+j] = xs[mi*128 + j, k*128 + p]
        xtiles.append(t)
    in_maps = []
    for c in range(DP * TP):
        dp, tp = divmod(c, TP)
        in_maps.append(
            {
                "xT": xtiles[dp],
                "ternT": np.ascontiguousarray(ternT[:, tp * O_c : (tp + 1) * O_c]),
                "scalesT": np.ascontiguousarray(
                    scalesT[:, tp * O_c : (tp + 1) * O_c]
                ),
            }
        )
    return in_maps


def kernel(x, ternary, scales, _trace=False):
    x = np.asarray(x, dtype=np.float32)
    ternary = np.asarray(ternary).astype(np.int8)  # {-1, 0, 1}
    scales = np.asarray(scales, dtype=np.float32)
    assert x.shape == (B, S, K) and ternary.shape == (O, K)

    if "nc" not in _nc_cache:
        _nc_cache["nc"] = _build_nc()
    nc = _nc_cache["nc"]

    in_maps = _host_prep(x.reshape(M, K), ternary, scales)
    res = run_bass_kernel_spmd(nc, in_maps, list(range(DP * TP)), trace=_trace)

    out2d = np.empty((M, O), np.float32)
    for c in range(DP * TP):
        dp, tp = divmod(c, TP)
        out2d[dp * M_c : (dp + 1) * M_c, tp * O_c : (tp + 1) * O_c] = res.results[
            c
        ]["out"]
    out = out2d.reshape(B, S, O)
    if _trace:
        return out, res.exec_time_ns
    return out
